# revision 18
# baseline (speedup 1.0000x reference)
"""Cross-attention (efficient-attention variant) + 1x1 conv + LayerNorm on 8 trn2 cores.

Problem: x1,x2 [4,64,64,1024] f32. Per batch b and head h (8 heads, 128 ch each):
  value = x1[b] channel-major, kq = x2[b] channel-major
  key = softmax(kq, tokens), query = softmax(kq, head-channels)
  S = query @ key^T  [128,128];  att = S @ value  -> agg [1024, 4096]
  y = w_proj[2048,1024] @ agg + b_proj; LayerNorm(2048) * gamma + beta

Sharding: core i -> batch b=i//2, token half i%2 (2048 tokens).

Reassociated projection: y^T = sum_h V_h^T @ G_h with G_h = S_h^T-contracted
wt_h, i.e. G_h[j,o] = sum_i S_h[i,j] wt[h*128+i, o].  Key-softmax normalizer
cs_h[j] = rowsum_i(S_raw_h) (exact because Q-hat rows sum to 1) is folded into
the G PSUM->SBUF drain as a per-partition scale.  This skips att entirely:
proj lhsT tiles are the DMA'd channel-major V directly.

LayerNorm of Wa (no bias) on device: S1/ssq via scalar-engine accum passes,
normalize split across vector (tensor_scalar) and scalar (Identity affine).
Bias/gamma/beta applied host-side as an exact affine fixup (b_proj==0 makes
it a pure gamma/beta scale); device also emits per-token (negmu, rsig).
"""

import os
import numpy as np

import concourse.bass as bass
import concourse.tile as tile
from concourse import bacc, mybir
from concourse.bass_utils import run_bass_kernel_spmd

F32 = mybir.dt.float32
BF16 = mybir.dt.bfloat16
AX = mybir.AxisListType
ALU = mybir.AluOpType
ACT_F = mybir.ActivationFunctionType

B, HI, WI, C = 4, 64, 64, 1024
N = HI * WI          # 4096 tokens per batch
HEADS = 8
CH = C // HEADS      # 128 per-head channels
C2 = 2 * C           # 2048 output channels
NCORES = 8
TOK = N // 2         # 2048 tokens per core
P = 128
NT_A = N // P        # 32 token tiles in phase A
NSUB = TOK // P      # 16 output subtiles in phase B
OC = C2 // 512       # output-channel chunks of 512
EPS = 1e-5

VH = int(os.environ.get("K_VH", "4"))    # heads of the Qhat mult on vector
GSC = int(os.environ.get("K_GSC", "4"))  # heads of the G drain on scalar
SPLIT = os.environ.get("K_SPLIT", "0") == "1"  # peer-split phase A + AllReduce
# (AllReduce crashes NRT in this runtime -- keep 0)
NT_LOC = NT_A // 2 if SPLIT else NT_A    # phase-A token tiles per core

_compiled = {}


def build():
    nc = bacc.Bacc("TRN2", target_bir_lowering=False, debug=False,
                   num_devices=NCORES)
    xq = nc.dram_tensor("xq", [NT_LOC * P, C], BF16,
                        kind="ExternalInput").ap()
    if SPLIT:
        sp_d = nc.dram_tensor("sp_bounce", [P, C], F32).ap()
        sf_d = nc.dram_tensor("sf_bounce", [P, C], F32).ap()
    vcm = nc.dram_tensor("vcm", [C, TOK], BF16, kind="ExternalInput").ap()
    wt = nc.dram_tensor("wt", [C, C2], BF16, kind="ExternalInput").ap()
    y = nc.dram_tensor("y", [TOK, C2], BF16, kind="ExternalOutput").ap()
    stats = nc.dram_tensor("stats", [P, 2 * NSUB], F32,
                           kind="ExternalOutput").ap()

    with tile.TileContext(nc) as tc:
        with tc.tile_pool(name="persist", bufs=1) as persist:
            eps_sb = persist.tile([P, 1], F32, name="eps")
            nc.vector.memset(eps_sb[:], EPS)
            ones_bf = persist.tile([P, 1], BF16, name="ones_bf")
            nc.vector.memset(ones_bf[:], 1.0)
            ones_f1 = persist.tile([1, 1], F32, name="ones_f1")
            nc.vector.memset(ones_f1[:], 1.0)
            wt_sb = [persist.tile([P, C2], BF16, name=f"wt{k}")
                     for k in range(HEADS)]
            vt_sb = [persist.tile([P, TOK], BF16, name=f"vt{h}")
                     for h in range(HEADS)]
            g_sb = [persist.tile([P, C2], BF16, name=f"g{h}")
                    for h in range(HEADS)]
            s_sb = [persist.tile([P, CH], BF16, name=f"s{h}")
                    for h in range(HEADS)]
            cs_sb = persist.tile([1, C], F32, name="cs_sb")
            cst_sb = persist.tile([P, HEADS], F32, name="cst_sb")
            rcs = persist.tile([P, HEADS], F32, name="rcs")
            stat_sb = persist.tile([P, 2 * NSUB], F32, name="stat_sb")
            scr1 = persist.tile([P, C2], BF16, name="scr1")
            scr2 = persist.tile([P, C2], BF16, name="scr2")

            # ---------------- Phase A: S_raw per head over all N tokens ------
            with tc.tile_pool(name="xq_p", bufs=4) as xq_p, \
                 tc.tile_pool(name="e_p", bufs=4) as e_p, \
                 tc.tile_pool(name="q_p", bufs=4) as q_p, \
                 tc.tile_pool(name="sm_a", bufs=8) as sm_a, \
                 tc.tile_pool(name="s_ps", bufs=1, space="PSUM") as s_psp:
                s_ps = s_psp.tile([P, C], F32, name="s_ps")
                for nt in range(NT_LOC):
                    xt = xq_p.tile([P, C], BF16)
                    nc.sync.dma_start(xt[:], xq[nt * P:(nt + 1) * P, :])
                    # spread the big phase-B input DMAs over early iterations
                    if nt < HEADS:
                        nc.sync.dma_start(wt_sb[nt][:],
                                          wt[nt * P:(nt + 1) * P, :])
                    elif nt < 2 * HEADS:
                        h = nt - HEADS
                        nc.sync.dma_start(vt_sb[h][:],
                                          vcm[h * P:(h + 1) * P, :])
                    E = e_p.tile([P, C], BF16)
                    nc.scalar.activation(E[:], xt[:], ACT_F.Exp)
                    qs = sm_a.tile([P, HEADS], BF16, name="qs")
                    with nc.allow_low_precision(
                            reason="softmax normalizer; residual absorbed by "
                                   "cs normalization"):
                        nc.vector.reduce_sum(
                            qs[:], E.rearrange("p (h c) -> p h c", h=HEADS),
                            axis=AX.X)
                    rq = sm_a.tile([P, HEADS], F32, name="rq")
                    nc.vector.reciprocal(rq[:], qs[:])
                    Qh = q_p.tile([P, C], BF16)
                    nc.vector.tensor_tensor(
                        Qh.rearrange("p (h c) -> p h c", h=HEADS)[:, :VH],
                        E.rearrange("p (h c) -> p h c", h=HEADS)[:, :VH],
                        rq[:, :VH, None].to_broadcast([P, VH, CH]),
                        op=ALU.mult)
                    if VH < HEADS:
                        nc.gpsimd.tensor_tensor(
                            Qh.rearrange("p (h c) -> p h c", h=HEADS)[:, VH:],
                            E.rearrange("p (h c) -> p h c", h=HEADS)[:, VH:],
                            rq[:, VH:, None].to_broadcast([P, HEADS - VH, CH]),
                            op=ALU.mult)
                    first, last = nt == 0, nt == NT_LOC - 1
                    for h in range(HEADS):
                        hs = slice(h * CH, (h + 1) * CH)
                        nc.tensor.matmul(s_ps[:, hs], lhsT=Qh[:, hs],
                                         rhs=E[:, hs], start=first, stop=last)
                if SPLIT:
                    # partial S -> DRAM -> pairwise AllReduce -> SBUF.  All
                    # transfers on the in-order gpsimd queue so the collective
                    # is correctly ordered w.r.t. its DRAM operands.
                    s_part = persist.tile([P, C], F32, name="s_part")
                    nc.scalar.copy(s_part[:], s_ps[:])
                    nc.gpsimd.dma_start(sp_d[:, :], s_part[:])
                    nc.gpsimd.collective_compute(
                        "AllReduce", ALU.add,
                        replica_groups=[[2 * i, 2 * i + 1]
                                        for i in range(NCORES // 2)],
                        ins=[sp_d[:, :]],
                        outs=[sf_d[:, :]])
                    s_full = persist.tile([P, C], F32, name="s_full")
                    nc.gpsimd.dma_start(s_full[:], sf_d[:, :])
                    for h in range(HEADS):
                        hs = slice(h * CH, (h + 1) * CH)
                        nc.scalar.copy(s_sb[h][:], s_full[:, hs])
                else:
                    # drain S_raw to SBUF (bf16) for use as matmul operands.
                    # all on scalar: vector is the phase-A pacer engine.
                    for h in range(HEADS):
                        hs = slice(h * CH, (h + 1) * CH)
                        nc.scalar.copy(s_sb[h][:], s_ps[:, hs])

            # ---------------- Bridge: cs, rcs, G ----------------------------
            with tc.tile_pool(name="cs_ps", bufs=1, space="PSUM") as cs_psp, \
                 tc.tile_pool(name="cst_ps", bufs=1, space="PSUM") as cst_psp:
                cs_ps = cs_psp.tile([1, C], F32, name="cs_ps")
                cst_ps = cst_psp.tile([P, HEADS], F32, name="cst_ps")
                for h in range(HEADS):
                    hs = slice(h * CH, (h + 1) * CH)
                    nc.tensor.matmul(cs_ps[:, hs], lhsT=ones_bf[:],
                                     rhs=s_sb[h][:], start=True, stop=True)
                    nc.scalar.copy(cs_sb[:, hs], cs_ps[:, hs])
                    nc.tensor.matmul(cst_ps[:, h:h + 1], lhsT=cs_sb[:, hs],
                                     rhs=ones_f1[:], start=True, stop=True)
                nc.scalar.copy(cst_sb[:], cst_ps[:])
                nc.vector.reciprocal(rcs[:], cst_sb[:])

            with tc.tile_pool(name="g_ps", bufs=2, space="PSUM") as g_psp:
                for h in range(HEADS):
                    g_ps = g_psp.tile([P, C2], F32, tag="g", name=f"g_ps{h}")
                    for oc in range(OC):
                        os_ = slice(oc * 512, (oc + 1) * 512)
                        nc.tensor.matmul(g_ps[:, os_], lhsT=s_sb[h][:],
                                         rhs=wt_sb[h][:, os_],
                                         start=True, stop=True)
                    # drain+scale by rcs (key-softmax normalizer), cast bf16
                    if h % 2 == 0 and GSC > 0:
                        nc.scalar.activation(g_sb[h][:], g_ps[:], ACT_F.Copy,
                                             scale=rcs[:, h:h + 1])
                    else:
                        nc.vector.tensor_scalar_mul(g_sb[h][:], g_ps[:],
                                                    rcs[:, h:h + 1])

            # ---------------- Phase B: proj + LayerNorm ----------------------
            with tc.tile_pool(name="y_ps", bufs=2, space="PSUM") as y_psp, \
                 tc.tile_pool(name="z_p", bufs=3) as z_p, \
                 tc.tile_pool(name="sm_b", bufs=10) as sm_b:
                for sub in range(NSUB):
                    ts = slice(sub * P, (sub + 1) * P)
                    yps = y_psp.tile([P, C2], F32, tag="y", name=f"yps{sub}")
                    for h in range(HEADS):
                        for oc in range(OC):
                            os_ = slice(oc * 512, (oc + 1) * 512)
                            nc.tensor.matmul(
                                yps[:, os_],
                                lhsT=vt_sb[h][:, ts],
                                rhs=g_sb[h][:, os_],
                                start=(h == 0), stop=(h == HEADS - 1))
                    # LayerNorm stats: alternate engines by subtile parity to
                    # balance load.  Even: scalar accum passes; odd: vector
                    # bn_stats (one pass for mean+var).
                    negmu = stat_sb[:, 2 * sub:2 * sub + 1]
                    var = sm_b.tile([P, 1], F32, name="var")
                    if sub % 2 == 0:
                        s1 = sm_b.tile([P, 1], F32, name="s1")
                        nc.scalar.activation(scr1[:], yps[:], ACT_F.Copy,
                                             accum_out=s1[:])
                        ssq = sm_b.tile([P, 1], F32, name="ssq")
                        nc.scalar.activation(scr2[:], yps[:], ACT_F.Square,
                                             accum_out=ssq[:])
                        nc.vector.tensor_scalar_mul(negmu, s1[:], -1.0 / C2)
                        m2 = sm_b.tile([P, 1], F32, name="m2")
                        nc.vector.tensor_tensor(m2[:], negmu, negmu,
                                                op=ALU.mult)
                        nc.vector.tensor_scalar(var[:], ssq[:], 1.0 / C2,
                                                m2[:], op0=ALU.mult,
                                                op1=ALU.subtract)
                    else:
                        bst = sm_b.tile([P, 4, 6], F32, name="bst")
                        for g in range(4):
                            nc.vector.bn_stats(
                                bst[:, g, :],
                                yps.rearrange("p (g x) -> p g x", g=4)[:, g])
                        mv = sm_b.tile([P, 2], F32, name="mv")
                        nc.vector.bn_aggr(mv[:], bst[:])
                        nc.vector.tensor_scalar_mul(negmu, mv[:, 0:1], -1.0)
                        nc.vector.tensor_copy(var[:], mv[:, 1:2])
                    sig = sm_b.tile([P, 1], F32, name="sig")
                    nc.scalar.activation(sig[:], var[:], ACT_F.Sqrt,
                                         bias=eps_sb[:])
                    rsig = stat_sb[:, 2 * sub + 1:2 * sub + 2]
                    nc.vector.reciprocal(rsig, sig[:])
                    nmrs = sm_b.tile([P, 1], F32, name="nmrs")
                    nc.vector.tensor_tensor(nmrs[:], negmu, rsig, op=ALU.mult)
                    # normalize: z = (Wa - mu)*rsig.  vector half (fused TS),
                    # scalar half (exact Copy-with-scale, then vector +nmrs on
                    # the fast all-SBUF path).
                    z = z_p.tile([P, C2], BF16)
                    HALF = C2 // 2
                    nc.vector.tensor_scalar(z[:, :HALF], yps[:, :HALF],
                                            negmu, rsig,
                                            op0=ALU.add, op1=ALU.mult)
                    nc.scalar.activation(z[:, HALF:], yps[:, HALF:],
                                         ACT_F.Copy, scale=rsig)
                    nc.vector.tensor_scalar_add(z[:, HALF:], z[:, HALF:],
                                                nmrs[:])
                    nc.sync.dma_start(y[ts, :], z[:])
                nc.sync.dma_start(stats[:], stat_sb[:])
    nc.compile()
    return nc


def _get_nc():
    if "nc" not in _compiled:
        _compiled["nc"] = build()
    return _compiled["nc"]


def run(inputs, trace=False):
    import ml_dtypes
    x1 = np.asarray(inputs["x1"], dtype=np.float32)
    x2 = np.asarray(inputs["x2"], dtype=np.float32)
    w_proj = np.asarray(inputs["w_proj"], dtype=np.float32)
    b_proj = np.asarray(inputs["b_proj"], dtype=np.float32)
    gamma = np.asarray(inputs["gamma"], dtype=np.float32)
    beta = np.asarray(inputs["beta"], dtype=np.float32)

    x1f = x1.reshape(B, N, C)
    x2f = x2.reshape(B, N, C).astype(ml_dtypes.bfloat16)
    wtp = np.ascontiguousarray(w_proj.T).astype(ml_dtypes.bfloat16)  # [C,2C]

    in_maps = []
    for core in range(NCORES):
        b, half = divmod(core, 2)
        vcm = np.ascontiguousarray(
            x1f[b].T[:, half * TOK:(half + 1) * TOK]).astype(ml_dtypes.bfloat16)
        xq_c = x2f[b][half * TOK:(half + 1) * TOK] if SPLIT else x2f[b]
        in_maps.append({
            "xq": np.ascontiguousarray(xq_c),
            "vcm": vcm,
            "wt": wtp,
        })
    nc = _get_nc()
    res = run_bass_kernel_spmd(nc, in_maps, list(range(NCORES)), trace=trace)

    zout = np.empty((B, N, C2), np.float32)
    negmu = np.empty((B, N), np.float32)
    rsig = np.empty((B, N), np.float32)
    for core in range(NCORES):
        b, half = divmod(core, 2)
        sl = slice(half * TOK, (half + 1) * TOK)
        zout[b, sl] = res.results[core]["y"].astype(np.float32)
        st = res.results[core]["stats"]  # [P, 2*NSUB]
        negmu[b, sl] = st[:, 0::2].T.reshape(TOK)
        rsig[b, sl] = st[:, 1::2].T.reshape(TOK)

    if np.any(b_proj):
        # exact affine fixup: device normalized Wa (no bias); redo LN stats
        # for Wa + b using z, negmu (=-mean(Wa)), rsig (=1/sqrt(var(Wa)+eps)).
        r0 = rsig.reshape(B, N, 1)
        mu_wa = -negmu.reshape(B, N, 1)
        mb = b_proj.mean()
        var_wa = 1.0 / r0**2 - EPS
        zb = np.einsum('bnc,c->bn', zout, b_proj)[..., None]
        wa_b = zb / r0 + mu_wa * b_proj.sum()
        cov = wa_b / C2 - mu_wa * mb
        var_y = var_wa + b_proj.var() + 2.0 * cov
        r_y = 1.0 / np.sqrt(var_y + EPS)
        out = (zout / r0 + (b_proj - mb)[None, None, :]) * r_y
        out = out * gamma + beta
    else:
        out = zout * gamma + beta
    return out.reshape(B, HI, WI, C2), res


def kernel(**inputs):
    out, _ = run(inputs, trace=False)
    return out


# revision 21
# speedup vs baseline: 1.0118x; 1.0118x over previous
"""Cross-attention (efficient-attention variant) + 1x1 conv + LayerNorm on 8 trn2 cores.

Problem: x1,x2 [4,64,64,1024] f32. Per batch b and head h (8 heads, 128 ch each):
  value = x1[b] channel-major, kq = x2[b] channel-major
  key = softmax(kq, tokens), query = softmax(kq, head-channels)
  S = query @ key^T  [128,128];  att = S @ value  -> agg [1024, 4096]
  y = w_proj[2048,1024] @ agg + b_proj; LayerNorm(2048) * gamma + beta

Sharding: core i -> batch b=i//2, token half i%2 (2048 tokens).

Reassociated projection: y^T = sum_h V_h^T @ G_h with G_h = S_h^T-contracted
wt_h, i.e. G_h[j,o] = sum_i S_h[i,j] wt[h*128+i, o].  Key-softmax normalizer
cs_h[j] = rowsum_i(S_raw_h) (exact because Q-hat rows sum to 1) is folded into
the G PSUM->SBUF drain as a per-partition scale.  This skips att entirely:
proj lhsT tiles are the DMA'd channel-major V directly.

LayerNorm of Wa (no bias) on device: S1/ssq via scalar-engine accum passes,
normalize split across vector (tensor_scalar) and scalar (Identity affine).
Bias/gamma/beta applied host-side as an exact affine fixup (b_proj==0 makes
it a pure gamma/beta scale); device also emits per-token (negmu, rsig).
"""

import os
import numpy as np

import concourse.bass as bass
import concourse.tile as tile
from concourse import bacc, mybir
from concourse.bass_utils import run_bass_kernel_spmd

F32 = mybir.dt.float32
BF16 = mybir.dt.bfloat16
AX = mybir.AxisListType
ALU = mybir.AluOpType
ACT_F = mybir.ActivationFunctionType

B, HI, WI, C = 4, 64, 64, 1024
N = HI * WI          # 4096 tokens per batch
HEADS = 8
CH = C // HEADS      # 128 per-head channels
C2 = 2 * C           # 2048 output channels
NCORES = 8
TOK = N // 2         # 2048 tokens per core
P = 128
NT_A = N // P        # 32 token tiles in phase A
NSUB = TOK // P      # 16 output subtiles in phase B
OC = C2 // 512       # output-channel chunks of 512
EPS = 1e-5

VH = int(os.environ.get("K_VH", "2"))    # heads of the Qhat mult on vector
GSC = int(os.environ.get("K_GSC", "4"))  # heads of the G drain on scalar
SPLIT = os.environ.get("K_SPLIT", "0") == "1"  # peer-split phase A + AllReduce
# (AllReduce crashes NRT in this runtime -- keep 0)
NT_LOC = NT_A // 2 if SPLIT else NT_A    # phase-A token tiles per core

_compiled = {}


def build():
    nc = bacc.Bacc("TRN2", target_bir_lowering=False, debug=False,
                   num_devices=NCORES)
    xq = nc.dram_tensor("xq", [NT_LOC * P, C], BF16,
                        kind="ExternalInput").ap()
    if SPLIT:
        sp_d = nc.dram_tensor("sp_bounce", [P, C], F32).ap()
        sf_d = nc.dram_tensor("sf_bounce", [P, C], F32).ap()
    vcm = nc.dram_tensor("vcm", [C, TOK], BF16, kind="ExternalInput").ap()
    wt = nc.dram_tensor("wt", [C, C2], BF16, kind="ExternalInput").ap()
    y = nc.dram_tensor("y", [TOK, C2], BF16, kind="ExternalOutput").ap()
    stats = nc.dram_tensor("stats", [P, 2 * NSUB], F32,
                           kind="ExternalOutput").ap()

    with tile.TileContext(nc) as tc:
        with tc.tile_pool(name="persist", bufs=1) as persist:
            eps_sb = persist.tile([P, 1], F32, name="eps")
            nc.vector.memset(eps_sb[:], EPS)
            ones_bf = persist.tile([P, 1], BF16, name="ones_bf")
            nc.vector.memset(ones_bf[:], 1.0)
            ones_f1 = persist.tile([1, 1], F32, name="ones_f1")
            nc.vector.memset(ones_f1[:], 1.0)
            wt_sb = [persist.tile([P, C2], BF16, name=f"wt{k}")
                     for k in range(HEADS)]
            vt_sb = [persist.tile([P, TOK], BF16, name=f"vt{h}")
                     for h in range(HEADS)]
            g_sb = [persist.tile([P, C2], BF16, name=f"g{h}")
                    for h in range(HEADS)]
            s_sb = [persist.tile([P, CH], BF16, name=f"s{h}")
                    for h in range(HEADS)]
            cs_sb = persist.tile([1, C], F32, name="cs_sb")
            cst_sb = persist.tile([P, HEADS], F32, name="cst_sb")
            rcs = persist.tile([P, HEADS], F32, name="rcs")
            stat_sb = persist.tile([P, 2 * NSUB], F32, name="stat_sb")
            scr1 = persist.tile([P, C2], BF16, name="scr1")
            scr2 = persist.tile([P, C2], BF16, name="scr2")

            # ---------------- Phase A: S_raw per head over all N tokens ------
            with tc.tile_pool(name="xq_p", bufs=4) as xq_p, \
                 tc.tile_pool(name="e_p", bufs=4) as e_p, \
                 tc.tile_pool(name="q_p", bufs=4) as q_p, \
                 tc.tile_pool(name="sm_a", bufs=8) as sm_a, \
                 tc.tile_pool(name="s_ps", bufs=1, space="PSUM") as s_psp:
                s_ps = s_psp.tile([P, C], F32, name="s_ps")
                for nt in range(NT_LOC):
                    xt = xq_p.tile([P, C], BF16)
                    nc.sync.dma_start(xt[:], xq[nt * P:(nt + 1) * P, :])
                    # spread the big phase-B input DMAs over early iterations
                    if nt < HEADS:
                        nc.sync.dma_start(wt_sb[nt][:],
                                          wt[nt * P:(nt + 1) * P, :])
                    elif nt < 2 * HEADS:
                        h = nt - HEADS
                        nc.sync.dma_start(vt_sb[h][:],
                                          vcm[h * P:(h + 1) * P, :])
                    E = e_p.tile([P, C], BF16)
                    nc.scalar.activation(E[:], xt[:], ACT_F.Exp)
                    qs = sm_a.tile([P, HEADS], BF16, name="qs")
                    with nc.allow_low_precision(
                            reason="softmax normalizer; residual absorbed by "
                                   "cs normalization"):
                        nc.vector.reduce_sum(
                            qs[:], E.rearrange("p (h c) -> p h c", h=HEADS),
                            axis=AX.X)
                    rq = sm_a.tile([P, HEADS], F32, name="rq")
                    nc.vector.reciprocal(rq[:], qs[:])
                    Qh = q_p.tile([P, C], BF16)
                    nc.vector.tensor_tensor(
                        Qh.rearrange("p (h c) -> p h c", h=HEADS)[:, :VH],
                        E.rearrange("p (h c) -> p h c", h=HEADS)[:, :VH],
                        rq[:, :VH, None].to_broadcast([P, VH, CH]),
                        op=ALU.mult)
                    if VH < HEADS:
                        nc.gpsimd.tensor_tensor(
                            Qh.rearrange("p (h c) -> p h c", h=HEADS)[:, VH:],
                            E.rearrange("p (h c) -> p h c", h=HEADS)[:, VH:],
                            rq[:, VH:, None].to_broadcast([P, HEADS - VH, CH]),
                            op=ALU.mult)
                    first, last = nt == 0, nt == NT_LOC - 1
                    for h in range(HEADS):
                        hs = slice(h * CH, (h + 1) * CH)
                        nc.tensor.matmul(s_ps[:, hs], lhsT=Qh[:, hs],
                                         rhs=E[:, hs], start=first, stop=last)
                if SPLIT:
                    # partial S -> DRAM -> pairwise AllReduce -> SBUF.  All
                    # transfers on the in-order gpsimd queue so the collective
                    # is correctly ordered w.r.t. its DRAM operands.
                    s_part = persist.tile([P, C], F32, name="s_part")
                    nc.scalar.copy(s_part[:], s_ps[:])
                    nc.gpsimd.dma_start(sp_d[:, :], s_part[:])
                    nc.gpsimd.collective_compute(
                        "AllReduce", ALU.add,
                        replica_groups=[[2 * i, 2 * i + 1]
                                        for i in range(NCORES // 2)],
                        ins=[sp_d[:, :]],
                        outs=[sf_d[:, :]])
                    s_full = persist.tile([P, C], F32, name="s_full")
                    nc.gpsimd.dma_start(s_full[:], sf_d[:, :])
                    for h in range(HEADS):
                        hs = slice(h * CH, (h + 1) * CH)
                        nc.scalar.copy(s_sb[h][:], s_full[:, hs])
                else:
                    # drain S_raw to SBUF (bf16) for use as matmul operands,
                    # split across scalar and vector to halve drain latency.
                    for h in range(HEADS):
                        hs = slice(h * CH, (h + 1) * CH)
                        if h % 2 == 0:
                            nc.scalar.copy(s_sb[h][:], s_ps[:, hs])
                        else:
                            nc.vector.tensor_copy(s_sb[h][:], s_ps[:, hs])

            # ---------------- Bridge: cs, rcs, G ----------------------------
            with tc.tile_pool(name="cs_ps", bufs=1, space="PSUM") as cs_psp, \
                 tc.tile_pool(name="cst_ps", bufs=1, space="PSUM") as cst_psp:
                cs_ps = cs_psp.tile([1, C], F32, name="cs_ps")
                cst_ps = cst_psp.tile([P, HEADS], F32, name="cst_ps")
                for h in range(HEADS):
                    hs = slice(h * CH, (h + 1) * CH)
                    nc.tensor.matmul(cs_ps[:, hs], lhsT=ones_bf[:],
                                     rhs=s_sb[h][:], start=True, stop=True)
                # two half copies so cst matmuls start before the full row
                # is drained (single-partition copies run on one DVE lane)
                nc.scalar.copy(cs_sb[:, :C // 2], cs_ps[:, :C // 2])
                nc.scalar.copy(cs_sb[:, C // 2:], cs_ps[:, C // 2:])
                for h in range(HEADS):
                    hs = slice(h * CH, (h + 1) * CH)
                    nc.tensor.matmul(cst_ps[:, h:h + 1], lhsT=cs_sb[:, hs],
                                     rhs=ones_f1[:], start=True, stop=True)
                nc.scalar.copy(cst_sb[:], cst_ps[:])
                nc.vector.reciprocal(rcs[:], cst_sb[:])

            with tc.tile_pool(name="g_ps", bufs=2, space="PSUM") as g_psp:
                for h in range(HEADS):
                    g_ps = g_psp.tile([P, C2], F32, tag="g", name=f"g_ps{h}")
                    for oc in range(OC):
                        os_ = slice(oc * 512, (oc + 1) * 512)
                        nc.tensor.matmul(g_ps[:, os_], lhsT=s_sb[h][:],
                                         rhs=wt_sb[h][:, os_],
                                         start=True, stop=True)
                    # drain+scale by rcs (key-softmax normalizer), cast bf16
                    if h % 2 == 0 and GSC > 0:
                        nc.scalar.activation(g_sb[h][:], g_ps[:], ACT_F.Copy,
                                             scale=rcs[:, h:h + 1])
                    else:
                        nc.vector.tensor_scalar_mul(g_sb[h][:], g_ps[:],
                                                    rcs[:, h:h + 1])

            # ---------------- Phase B: proj + LayerNorm ----------------------
            with tc.tile_pool(name="y_ps", bufs=2, space="PSUM") as y_psp, \
                 tc.tile_pool(name="z_p", bufs=3) as z_p, \
                 tc.tile_pool(name="sm_b", bufs=10) as sm_b:
                for sub in range(NSUB):
                    ts = slice(sub * P, (sub + 1) * P)
                    yps = y_psp.tile([P, C2], F32, tag="y", name=f"yps{sub}")
                    for h in range(HEADS):
                        for oc in range(OC):
                            os_ = slice(oc * 512, (oc + 1) * 512)
                            nc.tensor.matmul(
                                yps[:, os_],
                                lhsT=vt_sb[h][:, ts],
                                rhs=g_sb[h][:, os_],
                                start=(h == 0), stop=(h == HEADS - 1))
                    # LayerNorm stats: alternate engines by subtile parity to
                    # balance load.  Even: scalar accum passes; odd: vector
                    # bn_stats (one pass for mean+var).
                    negmu = stat_sb[:, 2 * sub:2 * sub + 1]
                    var = sm_b.tile([P, 1], F32, name="var")
                    if sub % 2 == 0:
                        s1 = sm_b.tile([P, 1], F32, name="s1")
                        nc.scalar.activation(scr1[:], yps[:], ACT_F.Copy,
                                             accum_out=s1[:])
                        ssq = sm_b.tile([P, 1], F32, name="ssq")
                        nc.scalar.activation(scr2[:], yps[:], ACT_F.Square,
                                             accum_out=ssq[:])
                        nc.vector.tensor_scalar_mul(negmu, s1[:], -1.0 / C2)
                        m2 = sm_b.tile([P, 1], F32, name="m2")
                        nc.vector.tensor_tensor(m2[:], negmu, negmu,
                                                op=ALU.mult)
                        nc.vector.tensor_scalar(var[:], ssq[:], 1.0 / C2,
                                                m2[:], op0=ALU.mult,
                                                op1=ALU.subtract)
                    else:
                        bst = sm_b.tile([P, 4, 6], F32, name="bst")
                        for g in range(4):
                            nc.vector.bn_stats(
                                bst[:, g, :],
                                yps.rearrange("p (g x) -> p g x", g=4)[:, g])
                        mv = sm_b.tile([P, 2], F32, name="mv")
                        nc.vector.bn_aggr(mv[:], bst[:])
                        nc.vector.tensor_scalar_mul(negmu, mv[:, 0:1], -1.0)
                        nc.vector.tensor_copy(var[:], mv[:, 1:2])
                    sig = sm_b.tile([P, 1], F32, name="sig")
                    nc.scalar.activation(sig[:], var[:], ACT_F.Sqrt,
                                         bias=eps_sb[:])
                    rsig = stat_sb[:, 2 * sub + 1:2 * sub + 2]
                    nc.vector.reciprocal(rsig, sig[:])
                    nmrs = sm_b.tile([P, 1], F32, name="nmrs")
                    nc.vector.tensor_tensor(nmrs[:], negmu, rsig, op=ALU.mult)
                    # normalize: z = (Wa - mu)*rsig.  vector half (fused TS),
                    # scalar half (exact Copy-with-scale, then vector +nmrs on
                    # the fast all-SBUF path).
                    z = z_p.tile([P, C2], BF16)
                    HALF = C2 // 2
                    nc.vector.tensor_scalar(z[:, :HALF], yps[:, :HALF],
                                            negmu, rsig,
                                            op0=ALU.add, op1=ALU.mult)
                    nc.scalar.activation(z[:, HALF:], yps[:, HALF:],
                                         ACT_F.Copy, scale=rsig)
                    nc.vector.tensor_scalar_add(z[:, HALF:], z[:, HALF:],
                                                nmrs[:])
                    nc.sync.dma_start(y[ts, :], z[:])
                nc.sync.dma_start(stats[:], stat_sb[:])
    nc.compile()
    return nc


def _get_nc():
    if "nc" not in _compiled:
        _compiled["nc"] = build()
    return _compiled["nc"]


def run(inputs, trace=False):
    import ml_dtypes
    x1 = np.asarray(inputs["x1"], dtype=np.float32)
    x2 = np.asarray(inputs["x2"], dtype=np.float32)
    w_proj = np.asarray(inputs["w_proj"], dtype=np.float32)
    b_proj = np.asarray(inputs["b_proj"], dtype=np.float32)
    gamma = np.asarray(inputs["gamma"], dtype=np.float32)
    beta = np.asarray(inputs["beta"], dtype=np.float32)

    x1f = x1.reshape(B, N, C)
    x2f = x2.reshape(B, N, C).astype(ml_dtypes.bfloat16)
    wtp = np.ascontiguousarray(w_proj.T).astype(ml_dtypes.bfloat16)  # [C,2C]

    in_maps = []
    for core in range(NCORES):
        b, half = divmod(core, 2)
        vcm = np.ascontiguousarray(
            x1f[b].T[:, half * TOK:(half + 1) * TOK]).astype(ml_dtypes.bfloat16)
        xq_c = x2f[b][half * TOK:(half + 1) * TOK] if SPLIT else x2f[b]
        in_maps.append({
            "xq": np.ascontiguousarray(xq_c),
            "vcm": vcm,
            "wt": wtp,
        })
    nc = _get_nc()
    res = run_bass_kernel_spmd(nc, in_maps, list(range(NCORES)), trace=trace)

    zout = np.empty((B, N, C2), np.float32)
    negmu = np.empty((B, N), np.float32)
    rsig = np.empty((B, N), np.float32)
    for core in range(NCORES):
        b, half = divmod(core, 2)
        sl = slice(half * TOK, (half + 1) * TOK)
        zout[b, sl] = res.results[core]["y"].astype(np.float32)
        st = res.results[core]["stats"]  # [P, 2*NSUB]
        negmu[b, sl] = st[:, 0::2].T.reshape(TOK)
        rsig[b, sl] = st[:, 1::2].T.reshape(TOK)

    if np.any(b_proj):
        # exact affine fixup: device normalized Wa (no bias); redo LN stats
        # for Wa + b using z, negmu (=-mean(Wa)), rsig (=1/sqrt(var(Wa)+eps)).
        r0 = rsig.reshape(B, N, 1)
        mu_wa = -negmu.reshape(B, N, 1)
        mb = b_proj.mean()
        var_wa = 1.0 / r0**2 - EPS
        zb = np.einsum('bnc,c->bn', zout, b_proj)[..., None]
        wa_b = zb / r0 + mu_wa * b_proj.sum()
        cov = wa_b / C2 - mu_wa * mb
        var_y = var_wa + b_proj.var() + 2.0 * cov
        r_y = 1.0 / np.sqrt(var_y + EPS)
        out = (zout / r0 + (b_proj - mb)[None, None, :]) * r_y
        out = out * gamma + beta
    else:
        out = zout * gamma + beta
    return out.reshape(B, HI, WI, C2), res


def kernel(**inputs):
    out, _ = run(inputs, trace=False)
    return out
